# revision 1
# baseline (speedup 1.0000x reference)
"""Bi-directional MinGRU kernel for Trainium2 (8 NeuronCores, SPMD).

Problem: x [4, 4096, 1024]; per direction d in {fwd, bwd}:
    k  = x @ Wz_d + bz_d
    A  = sigmoid(-k)           (= 1 - z, the carry coefficient)
    z  = sigmoid(k)
    gp = x @ Wh_d + bh_d
    g  = max(gp + 0.5, sigmoid(gp))      (== where(gp>=0, gp+0.5, sigmoid(gp)))
    h_t = A_t * h_{t-1} + z_t * g_t      (linear first-order scan over S)
    out = concat(h_fwd, h_bwd) @ W_out + b_out

Sharding: 8 cores = (4 batches) x (2 directions). Each core computes the
full hidden state for one (batch, direction) and its half of the final
2H->H projection; the two partial products per batch are summed on host.

Per-core layout: everything is kept transposed ([channel, seq]) so the
sequential scan runs along the free dimension with channels on partitions,
using the native VectorE tensor_tensor_scan instruction.
"""

import os
import numpy as np
from contextlib import ExitStack

import concourse.bass as bass
import concourse.tile as tile
from concourse import bacc, mybir
from concourse.bass_utils import run_bass_kernel_spmd

P = 128          # partitions
S = 4096         # sequence length
D = 1024         # input dim
H = 1024         # hidden dim
SC = 512         # seq chunk (one PSUM bank of fp32)
NSC = S // SC    # 8 seq chunks
ND = D // P      # 8 contraction tiles for GEMM1
NH = H // P      # 8 hidden tiles
NCORES = 8

F32 = mybir.dt.float32

# matmul input modes:
#   "f32r"   - all matmul inputs float32r (fp32 bytes, 1 cyc/row PE path)
#   "hybrid" - gate GEMMs (x, Wz, Wh) in bf16 (their error is damped by the
#              sigmoids), output GEMM (h, Wo) in float32r
#   "bf16"   - everything bf16
# float32r must be declared end-to-end (walrus birverifier requires the
# producer chain to be f32r-typed); the raw bytes are plain fp32.
# Default bf16: l2-rel 2.7e-3 => resid_var 7.2e-6, 14x under the
# concourse-standard gate (resid_var < 1e-4); fastest measured config.
MM_MODE = os.environ.get("BIMINGRU_MM_MODE", "bf16")

BF16 = mybir.dt.bfloat16
F32R = mybir.dt.float32r
if MM_MODE == "bf16":
    X_DT, O_DT = BF16, BF16
elif MM_MODE == "hybrid":
    X_DT, O_DT = BF16, F32R
elif MM_MODE == "f32r":
    X_DT, O_DT = F32R, F32R
else:
    X_DT, O_DT = F32, F32
H_DT = O_DT                      # scan output dtype (GEMM3 rhs)


def _np_dt(dt):
    if dt == BF16:
        import ml_dtypes
        return np.dtype(ml_dtypes.bfloat16)
    return np.dtype(np.float32)


def _mm(ap):
    return ap


def _build_module():
    nc = bacc.Bacc("TRN2", target_bir_lowering=False, debug=False)

    # All inputs are host-blocked so every SBUF working set is ONE contiguous
    # DMA (the sync engine's ~0.65us per-DMA issue cost dominates the ramp):
    #   xT row j*128+p, col d*512+c   = x^T[d*128+p, j*512+c]   (chunk-blocked)
    #   Wz/Wh row i*128+p, col d*128+c = W[d*128+p, i*128+c]    (i-blocked)
    #   Wo row o*128+p, col i*128+c    = W_half[i*128+p, o*128+c] (o-blocked)
    #   biasT [128, 4*NH] = [bz | -bz | bh | bh+0.5] per-partition columns
    xT = nc.dram_tensor("xT", [D, S], X_DT, kind="ExternalInput").ap()
    Wz = nc.dram_tensor("Wz", [D, H], X_DT, kind="ExternalInput").ap()
    Wh = nc.dram_tensor("Wh", [D, H], X_DT, kind="ExternalInput").ap()
    Wo = nc.dram_tensor("Wo", [H, H], O_DT, kind="ExternalInput").ap()
    biasT = nc.dram_tensor("biasT", [P, 4 * NH], F32, kind="ExternalInput").ap()
    outT = nc.dram_tensor("outT", [H, S], F32, kind="ExternalOutput").ap()

    AF = mybir.ActivationFunctionType
    OP = mybir.AluOpType

    with tile.TileContext(nc) as tc, ExitStack() as ctx:
        wpool = ctx.enter_context(tc.tile_pool(name="w", bufs=1))
        xpool = ctx.enter_context(tc.tile_pool(name="x", bufs=2))
        pspool = ctx.enter_context(tc.tile_pool(name="ps", bufs=2, space="PSUM"))
        ewpool = ctx.enter_context(tc.tile_pool(name="ew", bufs=2))
        hpool = ctx.enter_context(tc.tile_pool(name="h", bufs=2))
        opool = ctx.enter_context(tc.tile_pool(name="o", bufs=3))

        # --- PE warm-up: the first real matmul can't start until ~12us of
        # input DMA lands, and a cold PE then runs at 1.2GHz for another
        # ~3.4us (HAM). Burn that idle window with dummy matmuls on
        # memset-zero tiles so the HAM un-throttles before real work
        # arrives. The dummy PSUM tile reuses the psK tag (no extra bank).
        wdum = ewpool.tile([P, P], X_DT, tag="wdum", name="wdum")
        nc.vector.memset(wdum[:], 0)
        rdum = ewpool.tile([P, SC], X_DT, tag="rdum", name="rdum")
        nc.vector.memset(rdum[:], 0)
        # 10 dummies: they pace at ~0.33us each (WAW-serialized), so this
        # ends ~11.5us — right when HAM warms (8.1+3.4) and the first real
        # operands land; more would push the real stream start back.
        psdum = pspool.tile([P, SC], F32, tag="psK", bufs=3, name="psdum")
        for _ in range(10):
            nc.tensor.matmul(psdum[:], wdum[:], rdum[:], start=True, stop=True)

        x_chunks = {}

        def load_x_chunk(j):
            # one DMA per chunk: [128, ND*SC] with free dim (d, c)
            xt = xpool.tile([P, ND * SC], X_DT, tag="xb", name=f"xb_{j}")
            nc.sync.dma_start(xt[:], xT[j * P:(j + 1) * P, :])
            x_chunks[j] = xt

        # Startup: x chunk 0 is on the critical path to the first matmul, so
        # split it into 4 DMAs (parallel queues + per-MM wait granularity);
        # then the i-blocked Wz/Wh tiles interleaved — K(0,i) unblocks as
        # soon as block WzB[i] lands, so the PE ramps with the DMA stream.
        Wz_t, Wh_t, Wo_t = [], [], []
        xt0 = xpool.tile([P, ND * SC], X_DT, tag="xb", name="xb_0")
        QS = ND * SC // 4
        nc.sync.dma_start(xt0[:, 0:QS], xT[0:P, 0:QS])
        wzt = wpool.tile([P, H], X_DT, tag="wz0", name="wz0")
        nc.sync.dma_start(wzt[:], Wz[0:P, :])
        Wz_t.append(wzt)
        for q in range(1, 4):
            nc.sync.dma_start(xt0[:, q * QS:(q + 1) * QS],
                              xT[0:P, q * QS:(q + 1) * QS])
        x_chunks[0] = xt0

        bias_sb = wpool.tile([P, 4 * NH], F32, tag="bias", name="bias_sb")
        nc.sync.dma_start(bias_sb[:], biasT[:, :])
        bz_sb = bias_sb[:, 0:NH]
        nbz_sb = bias_sb[:, NH:2 * NH]
        bh_sb = bias_sb[:, 2 * NH:3 * NH]
        bh5_sb = bias_sb[:, 3 * NH:4 * NH]

        for i in range(1, NH):
            wzt = wpool.tile([P, H], X_DT, tag=f"wz{i}", name=f"wz{i}")
            nc.sync.dma_start(wzt[:], Wz[i * P:(i + 1) * P, :])
            Wz_t.append(wzt)
            wht = wpool.tile([P, H], X_DT, tag=f"wh{i-1}", name=f"wh{i-1}")
            nc.sync.dma_start(wht[:], Wh[(i - 1) * P:i * P, :])
            Wh_t.append(wht)
        wht = wpool.tile([P, H], X_DT, tag=f"wh{NH-1}", name=f"wh{NH-1}")
        nc.sync.dma_start(wht[:], Wh[(NH - 1) * P:NH * P, :])
        Wh_t.append(wht)

        def load_wo():
            for o in range(NH):
                wot = wpool.tile([P, H], O_DT, tag=f"wo{o}", name=f"wo{o}")
                nc.sync.dma_start(wot[:], Wo[o * P:(o + 1) * P, :])
                Wo_t.append(wot)

        h_tiles = [[None] * NH for _ in range(NSC)]

        stash = {}

        def emit_k(j, i):
            xc = x_chunks[j]
            psK = pspool.tile([P, SC], F32, tag="psK", bufs=3,
                              name=f"psK_{j}_{i}")
            for d in range(ND):
                nc.tensor.matmul(
                    psK[:], _mm(Wz_t[i][:, d * P:(d + 1) * P]),
                    _mm(xc[:, d * SC:(d + 1) * SC]),
                    start=(d == 0), stop=(d == ND - 1))
            A = ewpool.tile([P, SC], F32, tag="A", bufs=3, name=f"A_{j}_{i}")
            nc.scalar.activation(A[:], psK[:], AF.Sigmoid,
                                 bias=nbz_sb[:, i:i + 1], scale=-1.0)
            z = ewpool.tile([P, SC], F32, tag="z", bufs=3, name=f"z_{j}_{i}")
            nc.scalar.activation(z[:], psK[:], AF.Sigmoid,
                                 bias=bz_sb[:, i:i + 1], scale=1.0)
            stash[(j, i)] = (A, z)

        def emit_g(j, i):
            xc = x_chunks[j]
            psG = pspool.tile([P, SC], F32, tag="psG", bufs=3,
                              name=f"psG_{j}_{i}")
            for d in range(ND):
                nc.tensor.matmul(
                    psG[:], _mm(Wh_t[i][:, d * P:(d + 1) * P]),
                    _mm(xc[:, d * SC:(d + 1) * SC]),
                    start=(d == 0), stop=(d == ND - 1))
            A, z = stash.pop((j, i))
            sg = ewpool.tile([P, SC], F32, tag="sg", name=f"sg_{j}_{i}")
            nc.scalar.activation(sg[:], psG[:], AF.Sigmoid,
                                 bias=bh_sb[:, i:i + 1], scale=1.0)
            g = ewpool.tile([P, SC], F32, tag="g", name=f"g_{j}_{i}")
            nc.vector.scalar_tensor_tensor(g[:], psG[:], bh5_sb[:, i:i + 1],
                                           sg[:], op0=OP.add, op1=OP.max)
            Bv = ewpool.tile([P, SC], F32, tag="B", name=f"B_{j}_{i}")
            nc.vector.tensor_tensor(Bv[:], z[:], g[:], op=OP.mult)

            ht = hpool.tile([P, SC], H_DT, tag=f"h{i}", name=f"h_{j}_{i}")
            init = 0.0 if j == 0 else h_tiles[j - 1][i][:, SC - 1:SC]
            nc.vector.tensor_tensor_scan(ht[:], A[:], Bv[:], initial=init,
                                         op0=OP.mult, op1=OP.add)
            h_tiles[j][i] = ht

        def emit_o(j, o):
            psO = pspool.tile([P, SC], F32, tag="psO", name=f"psO_{j}_{o}")
            for i in range(NH):
                nc.tensor.matmul(
                    psO[:], _mm(Wo_t[o][:, i * P:(i + 1) * P]),
                    _mm(h_tiles[j][i][:]),
                    start=(i == 0), stop=(i == NH - 1))
            oc = opool.tile([P, SC], F32, tag="oc", name=f"oc_{j}_{o}")
            nc.scalar.copy(oc[:], psO[:])
            nc.sync.dma_start(outT[o * P:(o + 1) * P, j * SC:(j + 1) * SC], oc[:])

        # Software pipeline. Per chunk j the PE group order is
        #   K0 K1 [G0 O0] [K2 G1 O1] [K3 G2 O2] ... [K7 G6 O6] [G7 O7]
        # where O* are the GEMM3 groups of chunk j-1. Interleaving the O
        # groups keeps ~2 PE groups between G(i) and the DVE/ACT chain that
        # releases its PSUM bank, so the PE never stalls on the elementwise
        # tail. x(j+1) is prefetched at the head of chunk j; Wo loads are
        # issued at the head of chunk 1 (first needed by GEMM3 of chunk 0).
        for j in range(NSC):
            if j + 1 < NSC:
                load_x_chunk(j + 1)
            if j == 1:
                load_wo()
            emit_k(j, 0)
            emit_k(j, 1)
            for i in range(NH):
                if i + 2 < NH:
                    emit_k(j, i + 2)
                emit_g(j, i)
                if j >= 1:
                    emit_o(j - 1, i)
        for o in range(NH - 1):
            emit_o(NSC - 1, o)
        # final O group split into two N=256 halves so the first half's
        # copy+store overlaps the second half's matmuls (shorter serial
        # tail before the drain barrier); PSUM/SBUF tags are reused so no
        # extra banks are allocated
        j, o = NSC - 1, NH - 1
        HC = SC // 2
        for half in range(2):
            psO = pspool.tile([P, HC], F32, tag="psO", name=f"psOt_{half}")
            for i in range(NH):
                nc.tensor.matmul(
                    psO[:], _mm(Wo_t[o][:, i * P:(i + 1) * P]),
                    _mm(h_tiles[j][i][:, half * HC:(half + 1) * HC]),
                    start=(i == 0), stop=(i == NH - 1))
            oc = opool.tile([P, HC], F32, tag="oc", name=f"oct_{half}")
            nc.scalar.copy(oc[:], psO[:])
            nc.sync.dma_start(
                outT[o * P:(o + 1) * P,
                     j * SC + half * HC:j * SC + (half + 1) * HC], oc[:])

    nc.compile()
    return nc


_CACHE = {}


def _get_module():
    if "nc" not in _CACHE:
        _CACHE["nc"] = _build_module()
    return _CACHE["nc"]


def _make_in_maps(x, Wz_f, bz_f, Wh_f, bh_f, Wz_b, bz_b, Wh_b, bh_b, W_out, b_out):
    np_x = _np_dt(X_DT)
    np_o = _np_dt(O_DT)
    f32 = np.float32

    def blk_w(w, dt):
        # [D, H] -> blocked: out[i*128+p, d*128+c] = w[d*128+p, i*128+c]
        w = np.asarray(w, dtype=f32).reshape(ND, P, NH, P)
        return np.ascontiguousarray(
            w.transpose(2, 1, 0, 3).reshape(H, D), dtype=dt)

    def blk_x(xb, rev):
        # [S, D] -> blocked: out[j*128+p, d*512+c] = x[j*512+c, d*128+p]
        if rev:
            xb = xb[::-1]
        xb = xb.reshape(NSC, SC, ND, P)
        return np.ascontiguousarray(
            xb.transpose(0, 3, 2, 1).reshape(NSC * P, ND * SC), dtype=np_x)

    x = np.asarray(x, dtype=f32)
    Wz_fc, Wh_fc = blk_w(Wz_f, np_x), blk_w(Wh_f, np_x)
    Wz_bc, Wh_bc = blk_w(Wz_b, np_x), blk_w(Wh_b, np_x)
    W_out = np.asarray(W_out)
    Wo_fc = blk_w(W_out[:H], np_o)      # fwd half rows of W_out
    Wo_bc = blk_w(W_out[H:], np_o)      # bwd half rows

    def bias_pack(b_z, b_h):
        def col(v):  # [H] -> [128, NH] with col i = h-tile i
            return np.asarray(v, dtype=f32).reshape(NH, P).T
        b_z = np.asarray(b_z, dtype=f32)
        b_h = np.asarray(b_h, dtype=f32)
        return {"biasT": np.ascontiguousarray(np.concatenate(
            [col(b_z), col(-b_z), col(b_h), col(b_h + 0.5)], axis=1))}

    bias_f = bias_pack(bz_f, bh_f)
    bias_b = bias_pack(bz_b, bh_b)

    in_maps = []
    for b in range(4):
        xT_f = blk_x(x[b], rev=False)
        xT_b = blk_x(x[b], rev=True)
        in_maps.append({"xT": xT_f, "Wz": Wz_fc, "Wh": Wh_fc, "Wo": Wo_fc,
                        **bias_f})
        in_maps.append({"xT": xT_b, "Wz": Wz_bc, "Wh": Wh_bc, "Wo": Wo_bc,
                        **bias_b})
    return in_maps


def _assemble(results, b_out):
    out = np.empty((4, S, H), np.float32)
    for b in range(4):
        out[b] = results[2 * b]["outT"].T
        out[b] += results[2 * b + 1]["outT"].T
    out += np.asarray(b_out, dtype=np.float32)
    return out


def kernel(x, Wz_f, bz_f, Wh_f, bh_f, Wz_b, bz_b, Wh_b, bh_b, W_out, b_out):
    nc = _get_module()
    in_maps = _make_in_maps(x, Wz_f, bz_f, Wh_f, bh_f,
                            Wz_b, bz_b, Wh_b, bh_b, W_out, b_out)
    res = run_bass_kernel_spmd(nc, in_maps, core_ids=list(range(NCORES)))
    return _assemble(res.results, b_out)



# revision 4
# speedup vs baseline: 1.2285x; 1.2285x over previous
"""Bi-directional MinGRU kernel for Trainium2 (8 NeuronCores, SPMD).

Problem: x [4, 4096, 1024]; per direction d in {fwd, bwd}:
    k  = x @ Wz_d + bz_d
    A  = sigmoid(-k)           (= 1 - z, the carry coefficient)
    z  = sigmoid(k)
    gp = x @ Wh_d + bh_d
    g  = max(gp + 0.5, sigmoid(gp))      (== where(gp>=0, gp+0.5, sigmoid(gp)))
    h_t = A_t * h_{t-1} + z_t * g_t      (linear first-order scan over S)
    out = concat(h_fwd, h_bwd) @ W_out + b_out

Sharding: 8 cores = (4 batches) x (2 directions). Each core computes the
full hidden state for one (batch, direction) and its half of the final
2H->H projection; the two partial products per batch are summed on host.

Per-core layout: everything is kept transposed ([channel, seq]) so the
sequential scan runs along the free dimension with channels on partitions,
using the native VectorE tensor_tensor_scan instruction.

Precision plan (error budget, gate rel < 2e-2; all rels are exact
host-side replications of the device arithmetic, see fp8_sweep.py):
  - z-GEMM (k = x@Wz) fully fp8-e4m3 in DoubleRow mode (2 k-subtiles per
    matmul at ~1.77x bf16 rate). The z errors are strongly damped: the
    sigmoid slope (<=1/4) and the scan innovation form dh = dz*(g - h)
    cancel most of it. Contribution ~8.5e-3.
  - h-GEMM (gp = x@Wh): H8P k-subtile PAIRS in fp8 DoubleRow, the rest
    bf16, accumulated into the same PSUM tile. g-errors pass slope-1
    through max(gp+.5, sg), so only a fraction of K may be fp8.
  - out-GEMM (h@Wo) stays bf16: its error hits the output undamped
    (fp8 would contribute ~4.1e-2 alone).
  - Wz is host-scaled by 32 (=> fewer e4m3 subnormal flushes); folded
    back via the ACT sigmoid scale parameter. x and Wh stay unscaled so
    the shared x8 tile and the un-scaled psG->g DVE path work unchanged.
"""

import os
import numpy as np
from contextlib import ExitStack

import concourse.bass as bass
import concourse.tile as tile
from concourse import bacc, mybir
from concourse.bass_utils import run_bass_kernel_spmd

P = 128          # partitions
S = 4096         # sequence length
D = 1024         # input dim
H = 1024         # hidden dim
SC = 512         # seq chunk (one PSUM bank of fp32)
NSC = S // SC    # 8 seq chunks
ND = D // P      # 8 contraction subtiles for GEMM1
NH = H // P      # 8 hidden tiles
NCORES = 8

# h-GEMM fp8 subtile-pairs (each pair = 2 of the ND=8 k-subtiles).
H8P = int(os.environ.get("BIMINGRU_H8P", "1"))
NB = ND - 2 * H8P            # bf16 k-subtiles for the h-GEMM
WZ_SCALE = 32.0

F32 = mybir.dt.float32
BF16 = mybir.dt.bfloat16
F8 = mybir.dt.float8e4
O_DT = BF16                  # out-GEMM operand dtype
H_DT = BF16                  # scan output dtype (GEMM3 rhs)
DR = mybir.MatmulPerfMode.DoubleRow


def _np_dt(dt):
    import ml_dtypes
    if dt == BF16:
        return np.dtype(ml_dtypes.bfloat16)
    if dt == F8:
        return np.dtype(ml_dtypes.float8_e4m3)
    return np.dtype(np.float32)


def _build_module():
    nc = bacc.Bacc("TRN2", target_bir_lowering=False, debug=False)

    # Host-blocked inputs; one contiguous DMA per SBUF working set (the
    # sync engine's ~0.65us per-DMA issue cost would dominate the ramp):
    #   xT8 row j*128+p, col d*512+c  = x^T[d*128+p, j*512+c]   (fp8)
    #   xT16 row j*128+p, col b*512+c = x^T[(2*H8P+b)*128+p, j*512+c]
    #   Wz8  [128, NH*ND*128] fp8: [p, i*1024+d*128+c] = 32*Wz[d*128+p, i*128+c]
    #   Wh8  [128, NH*2*H8P*128] fp8: [p, i*2*H8P*128+q*128+c] = Wh[q*128+p, i*128+c]
    #   Wh16 [128, NH*NB*128] bf16: [p, i*NB*128+b*128+c] = Wh[(2*H8P+b)*128+p, i*128+c]
    #   Wo   [128, NH*H] bf16: [p, o*1024+i*128+c] = W_half[i*128+p, o*128+c]
    #   biasT [128, 4*NH] = [bz | -bz | bh | bh+0.5] per-partition columns
    xT8 = nc.dram_tensor("xT8", [NSC * P, ND * SC], F8, kind="ExternalInput").ap()
    Wz8 = nc.dram_tensor("Wz8", [P, NH * ND * P], F8, kind="ExternalInput").ap()
    if H8P > 0:
        Wh8 = nc.dram_tensor("Wh8", [P, NH * 2 * H8P * P], F8,
                             kind="ExternalInput").ap()
    if NB > 0:
        xT16 = nc.dram_tensor("xT16", [NSC * P, NB * SC], BF16,
                              kind="ExternalInput").ap()
        Wh16 = nc.dram_tensor("Wh16", [P, NH * NB * P], BF16,
                              kind="ExternalInput").ap()
    Wo = nc.dram_tensor("Wo", [P, NH * H], O_DT, kind="ExternalInput").ap()
    biasT = nc.dram_tensor("biasT", [P, 4 * NH], F32, kind="ExternalInput").ap()
    outT = nc.dram_tensor("outT", [H, S], F32, kind="ExternalOutput").ap()

    AF = mybir.ActivationFunctionType
    OP = mybir.AluOpType

    with tile.TileContext(nc) as tc, ExitStack() as ctx:
        wpool = ctx.enter_context(tc.tile_pool(name="w", bufs=1))
        xpool = ctx.enter_context(tc.tile_pool(name="x", bufs=2))
        pspool = ctx.enter_context(tc.tile_pool(name="ps", bufs=2, space="PSUM"))
        ewpool = ctx.enter_context(tc.tile_pool(name="ew", bufs=2))
        hpool = ctx.enter_context(tc.tile_pool(name="h", bufs=2))
        opool = ctx.enter_context(tc.tile_pool(name="o", bufs=3))

        # --- PE warm-up: dummy matmuls keep the PE HAM busy from t~0 so
        # the clock un-throttles (1.2 -> 2.4 GHz needs ~3.4us of sustained
        # activity) before the first real operands land (~6us).
        wdum = ewpool.tile([P, P], BF16, tag="wdum", name="wdum")
        nc.vector.memset(wdum[:], 0)
        rdum = ewpool.tile([P, SC], BF16, tag="rdum", name="rdum")
        nc.vector.memset(rdum[:], 0)
        psdum = pspool.tile([P, SC], F32, tag="psK", bufs=3, name="psdum")
        for _ in range(14):
            nc.tensor.matmul(psdum[:], wdum[:], rdum[:], start=True, stop=True)

        x8_chunks = {}
        x16_chunks = {}

        def load_x_chunk(j, split=1):
            xt8 = xpool.tile([P, ND * SC], F8, tag="x8", name=f"x8_{j}")
            qs = ND * SC // split
            for q in range(split):
                nc.sync.dma_start(xt8[:, q * qs:(q + 1) * qs],
                                  xT8[j * P:(j + 1) * P, q * qs:(q + 1) * qs])
            x8_chunks[j] = xt8
            if NB > 0:
                xt16 = xpool.tile([P, NB * SC], BF16, tag="x16", name=f"x16_{j}")
                nc.sync.dma_start(xt16[:], xT16[j * P:(j + 1) * P, :])
                x16_chunks[j] = xt16

        # Startup order: x8 chunk 0 + Wz8 are the critical path to the
        # first matmul; then bias, x16 chunk 0 + Wh for the G groups,
        # then Wo (first needed by O(0,0), ~4 i-steps into chunk 0).
        xt8 = xpool.tile([P, ND * SC], F8, tag="x8", name="x8_0")
        qs = ND * SC // 2
        nc.sync.dma_start(xt8[:, 0:qs], xT8[0:P, 0:qs])
        wz8_sb = wpool.tile([P, NH * ND * P], F8, tag="wz8", name="wz8_sb")
        nc.sync.dma_start(wz8_sb[:, 0:NH * ND * P // 2],
                          Wz8[:, 0:NH * ND * P // 2])
        nc.sync.dma_start(xt8[:, qs:2 * qs], xT8[0:P, qs:2 * qs])
        nc.sync.dma_start(wz8_sb[:, NH * ND * P // 2:],
                          Wz8[:, NH * ND * P // 2:])
        x8_chunks[0] = xt8

        bias_sb = wpool.tile([P, 4 * NH], F32, tag="bias", name="bias_sb")
        nc.sync.dma_start(bias_sb[:], biasT[:, :])
        bz_sb = bias_sb[:, 0:NH]
        nbz_sb = bias_sb[:, NH:2 * NH]
        bh_sb = bias_sb[:, 2 * NH:3 * NH]
        bh5_sb = bias_sb[:, 3 * NH:4 * NH]

        if NB > 0:
            xt16 = xpool.tile([P, NB * SC], BF16, tag="x16", name="x16_0")
            nc.sync.dma_start(xt16[:], xT16[0:P, :])
            x16_chunks[0] = xt16
        if H8P > 0:
            wh8_sb = wpool.tile([P, NH * 2 * H8P * P], F8, tag="wh8",
                                name="wh8_sb")
            nc.sync.dma_start(wh8_sb[:], Wh8[:, :])
        if NB > 0:
            wh16_sb = wpool.tile([P, NH * NB * P], BF16, tag="wh16",
                                 name="wh16_sb")
            half = NH * NB * P // 2
            nc.sync.dma_start(wh16_sb[:, 0:half], Wh16[:, 0:half])
            nc.sync.dma_start(wh16_sb[:, half:], Wh16[:, half:])

        wo_sb = wpool.tile([P, NH * H], O_DT, tag="wo", name="wo_sb")

        def load_wo():
            half = NH * H // 2
            nc.sync.dma_start(wo_sb[:, 0:half], Wo[:, 0:half])
            nc.sync.dma_start(wo_sb[:, half:], Wo[:, half:])

        # per-i weight views
        wz8_v = [wz8_sb[:, i * ND * P:(i + 1) * ND * P]
                 .rearrange("p (d c) -> p d c", c=P) for i in range(NH)]
        if H8P > 0:
            wh8_v = [wh8_sb[:, i * 2 * H8P * P:(i + 1) * 2 * H8P * P]
                     .rearrange("p (d c) -> p d c", c=P) for i in range(NH)]
        if NB > 0:
            wh16_v = [wh16_sb[:, i * NB * P:(i + 1) * NB * P]
                      for i in range(NH)]
        wo_v = [wo_sb[:, o * H:(o + 1) * H] for o in range(NH)]

        h_tiles = [[None] * NH for _ in range(NSC)]
        stash = {}

        def emit_k(j, i):
            x8v = x8_chunks[j].rearrange("p (d c) -> p d c", c=SC)
            psK = pspool.tile([P, SC], F32, tag="psK", bufs=3,
                              name=f"psK_{j}_{i}")
            for q in range(ND // 2):
                nc.tensor.matmul(
                    psK[:], wz8_v[i][:, 2 * q:2 * q + 2, :],
                    x8v[:, 2 * q:2 * q + 2, :],
                    start=(q == 0), stop=(q == ND // 2 - 1), perf_mode=DR)
            A = ewpool.tile([P, SC], F32, tag="A", bufs=3, name=f"A_{j}_{i}")
            nc.scalar.activation(A[:], psK[:], AF.Sigmoid,
                                 bias=nbz_sb[:, i:i + 1], scale=-1.0 / WZ_SCALE)
            z = ewpool.tile([P, SC], F32, tag="z", bufs=3, name=f"z_{j}_{i}")
            nc.scalar.activation(z[:], psK[:], AF.Sigmoid,
                                 bias=bz_sb[:, i:i + 1], scale=1.0 / WZ_SCALE)
            stash[(j, i)] = (A, z)

        def emit_g(j, i):
            psG = pspool.tile([P, SC], F32, tag="psG", bufs=3,
                              name=f"psG_{j}_{i}")
            if H8P > 0:
                x8v = x8_chunks[j].rearrange("p (d c) -> p d c", c=SC)
                for q in range(H8P):
                    nc.tensor.matmul(
                        psG[:], wh8_v[i][:, 2 * q:2 * q + 2, :],
                        x8v[:, 2 * q:2 * q + 2, :],
                        start=(q == 0), stop=(NB == 0 and q == H8P - 1),
                        perf_mode=DR)
            if NB > 0:
                x16c = x16_chunks[j]
                for b in range(NB):
                    nc.tensor.matmul(
                        psG[:], wh16_v[i][:, b * P:(b + 1) * P],
                        x16c[:, b * SC:(b + 1) * SC],
                        start=(H8P == 0 and b == 0), stop=(b == NB - 1))
            A, z = stash.pop((j, i))
            sg = ewpool.tile([P, SC], F32, tag="sg", name=f"sg_{j}_{i}")
            nc.scalar.activation(sg[:], psG[:], AF.Sigmoid,
                                 bias=bh_sb[:, i:i + 1], scale=1.0)
            g = ewpool.tile([P, SC], F32, tag="g", name=f"g_{j}_{i}")
            nc.vector.scalar_tensor_tensor(g[:], psG[:], bh5_sb[:, i:i + 1],
                                           sg[:], op0=OP.add, op1=OP.max)
            Bv = ewpool.tile([P, SC], F32, tag="B", name=f"B_{j}_{i}")
            nc.vector.tensor_tensor(Bv[:], z[:], g[:], op=OP.mult)

            ht = hpool.tile([P, SC], H_DT, tag=f"h{i}", name=f"h_{j}_{i}")
            init = 0.0 if j == 0 else h_tiles[j - 1][i][:, SC - 1:SC]
            nc.vector.tensor_tensor_scan(ht[:], A[:], Bv[:], initial=init,
                                         op0=OP.mult, op1=OP.add)
            h_tiles[j][i] = ht

        def emit_o(j, o):
            psO = pspool.tile([P, SC], F32, tag="psO", name=f"psO_{j}_{o}")
            for i in range(NH):
                nc.tensor.matmul(
                    psO[:], wo_v[o][:, i * P:(i + 1) * P],
                    h_tiles[j][i][:],
                    start=(i == 0), stop=(i == NH - 1))
            oc = opool.tile([P, SC], F32, tag="oc", name=f"oc_{j}_{o}")
            nc.scalar.copy(oc[:], psO[:])
            nc.sync.dma_start(outT[o * P:(o + 1) * P, j * SC:(j + 1) * SC], oc[:])

        # Software pipeline. Per chunk j the PE group order is
        #   K0 K1 [G0 O0] [K2 G1 O1] ... [K7 G6 O6] [G7 O7]
        # where O* are the GEMM3 groups of chunk j-1 (GEMM3 contracts over
        # all NH h-tiles, so it can only start after the whole previous
        # chunk's scans). x(j+1) is prefetched at the head of chunk j.
        for j in range(NSC):
            if j == 0:
                load_wo()
            if j + 1 < NSC:
                load_x_chunk(j + 1)
            emit_k(j, 0)
            emit_k(j, 1)
            for i in range(NH):
                if i + 2 < NH:
                    emit_k(j, i + 2)
                emit_g(j, i)
                if j >= 1:
                    emit_o(j - 1, i)
        for o in range(NH - 1):
            emit_o(NSC - 1, o)
        # final O group split into two N=256 halves so the first half's
        # copy+store overlaps the second half's matmuls (shorter serial
        # tail before the drain barrier)
        j, o = NSC - 1, NH - 1
        HC = SC // 2
        for half in range(2):
            psO = pspool.tile([P, HC], F32, tag="psO", name=f"psOt_{half}")
            for i in range(NH):
                nc.tensor.matmul(
                    psO[:], wo_v[o][:, i * P:(i + 1) * P],
                    h_tiles[j][i][:, half * HC:(half + 1) * HC],
                    start=(i == 0), stop=(i == NH - 1))
            oc = opool.tile([P, HC], F32, tag="oc", name=f"oct_{half}")
            nc.scalar.copy(oc[:], psO[:])
            nc.sync.dma_start(
                outT[o * P:(o + 1) * P,
                     j * SC + half * HC:j * SC + (half + 1) * HC], oc[:])

    nc.compile()
    return nc


_CACHE = {}


def _get_module():
    if "nc" not in _CACHE:
        _CACHE["nc"] = _build_module()
    return _CACHE["nc"]


def _make_in_maps(x, Wz_f, bz_f, Wh_f, bh_f, Wz_b, bz_b, Wh_b, bh_b, W_out, b_out):
    np_f8 = _np_dt(F8)
    np_bf = _np_dt(BF16)
    np_o = _np_dt(O_DT)
    f32 = np.float32

    def blk_w(w, dt, scale=1.0):
        # [D, H] -> [128, NH*ND*128]: out[p, i*D + d*128 + c] = w[d*128+p, i*128+c]
        w = np.asarray(w, dtype=f32)
        if scale != 1.0:
            w = w * scale
        w = w.reshape(ND, P, NH, P)
        # target axes order: (p, i, d, c)
        return np.ascontiguousarray(
            w.transpose(1, 2, 0, 3).reshape(P, NH * ND * P), dtype=dt)

    def blk_w_rows(w, dt, d_lo, d_hi):
        # rows d_lo*128..d_hi*128 of [D, H] -> [128, NH*(d_hi-d_lo)*128]
        nd = d_hi - d_lo
        w = np.asarray(w[d_lo * P:d_hi * P], dtype=f32).reshape(nd, P, NH, P)
        return np.ascontiguousarray(
            w.transpose(1, 2, 0, 3).reshape(P, NH * nd * P), dtype=dt)

    def blk_x(xb, rev, d_lo, d_hi, dt):
        # [S, D] cols d_lo*128..d_hi*128 -> [NSC*128, nd*512]:
        # out[j*128+p, b*512+c] = x[j*512+c, (d_lo+b)*128+p]
        nd = d_hi - d_lo
        if rev:
            xb = xb[::-1]
        xb = xb[:, d_lo * P:d_hi * P].reshape(NSC, SC, nd, P)
        return np.ascontiguousarray(
            xb.transpose(0, 3, 2, 1).reshape(NSC * P, nd * SC), dtype=dt)

    def blk_wo(w, dt):
        # [H, H] -> [128, NH*H]: out[p, o*H + i*128 + c] = w[i*128+p, o*128+c]
        w = np.asarray(w, dtype=f32).reshape(NH, P, NH, P)
        return np.ascontiguousarray(
            w.transpose(1, 2, 0, 3).reshape(P, NH * H), dtype=dt)

    x = np.asarray(x, dtype=f32)
    W_out = np.asarray(W_out)

    def weights(Wz, Wh):
        m = {"Wz8": blk_w(Wz, np_f8, scale=WZ_SCALE)}
        if H8P > 0:
            m["Wh8"] = blk_w_rows(Wh, np_f8, 0, 2 * H8P)
        if NB > 0:
            m["Wh16"] = blk_w_rows(Wh, np_bf, 2 * H8P, ND)
        return m

    w_f = weights(Wz_f, Wh_f)
    w_b = weights(Wz_b, Wh_b)
    wo_f = blk_wo(W_out[:H], np_o)      # fwd half rows of W_out
    wo_b = blk_wo(W_out[H:], np_o)      # bwd half rows

    def bias_pack(b_z, b_h):
        def col(v):  # [H] -> [128, NH] with col i = h-tile i
            return np.asarray(v, dtype=f32).reshape(NH, P).T
        b_z = np.asarray(b_z, dtype=f32)
        b_h = np.asarray(b_h, dtype=f32)
        return np.ascontiguousarray(np.concatenate(
            [col(b_z), col(-b_z), col(b_h), col(b_h + 0.5)], axis=1))

    bias_f = bias_pack(bz_f, bh_f)
    bias_b = bias_pack(bz_b, bh_b)

    in_maps = []
    for b in range(4):
        for rev, wd, wo, bias in ((False, w_f, wo_f, bias_f),
                                  (True, w_b, wo_b, bias_b)):
            m = {"xT8": blk_x(x[b], rev, 0, ND, np_f8),
                 "Wo": wo, "biasT": bias, **wd}
            if NB > 0:
                m["xT16"] = blk_x(x[b], rev, 2 * H8P, ND, np_bf)
            in_maps.append(m)
    return in_maps


def _assemble(results, b_out):
    out = np.empty((4, S, H), np.float32)
    for b in range(4):
        out[b] = results[2 * b]["outT"].T
        out[b] += results[2 * b + 1]["outT"].T
    out += np.asarray(b_out, dtype=np.float32)
    return out


def kernel(x, Wz_f, bz_f, Wh_f, bh_f, Wz_b, bz_b, Wh_b, bh_b, W_out, b_out):
    nc = _get_module()
    in_maps = _make_in_maps(x, Wz_f, bz_f, Wh_f, bh_f,
                            Wz_b, bz_b, Wh_b, bh_b, W_out, b_out)
    res = run_bass_kernel_spmd(nc, in_maps, core_ids=list(range(NCORES)))
    return _assemble(res.results, b_out)


# revision 8
# speedup vs baseline: 1.2306x; 1.0017x over previous
"""Bi-directional MinGRU kernel for Trainium2 (8 NeuronCores, SPMD).

Problem: x [4, 4096, 1024]; per direction d in {fwd, bwd}:
    k  = x @ Wz_d + bz_d
    A  = sigmoid(-k)           (= 1 - z, the carry coefficient)
    z  = sigmoid(k)
    gp = x @ Wh_d + bh_d
    g  = max(gp + 0.5, sigmoid(gp))      (== where(gp>=0, gp+0.5, sigmoid(gp)))
    h_t = A_t * h_{t-1} + z_t * g_t      (linear first-order scan over S)
    out = concat(h_fwd, h_bwd) @ W_out + b_out

Sharding: 8 cores = (4 batches) x (2 directions). Each core computes the
full hidden state for one (batch, direction) and its half of the final
2H->H projection; the two partial products per batch are summed on host.

Per-core layout: everything is kept transposed ([channel, seq]) so the
sequential scan runs along the free dimension with channels on partitions,
using the native VectorE tensor_tensor_scan instruction.

Precision plan (error budget, gate rel < 2e-2; all rels are exact
host-side replications of the device arithmetic, see fp8_sweep.py):
  - z-GEMM (k = x@Wz) fully fp8-e4m3 in DoubleRow mode (2 k-subtiles per
    matmul at ~1.77x bf16 rate). The z errors are strongly damped: the
    sigmoid slope (<=1/4) and the scan innovation form dh = dz*(g - h)
    cancel most of it. Contribution ~8.5e-3.
  - h-GEMM (gp = x@Wh): H8P k-subtile PAIRS in fp8 DoubleRow, the rest
    bf16, accumulated into the same PSUM tile. g-errors pass slope-1
    through max(gp+.5, sg), so only a fraction of K may be fp8.
  - out-GEMM (h@Wo) stays bf16: its error hits the output undamped
    (fp8 would contribute ~4.1e-2 alone).
  - Wz is host-scaled by 32 (=> fewer e4m3 subnormal flushes); folded
    back via the ACT sigmoid scale parameter. x and Wh stay unscaled so
    the shared x8 tile and the un-scaled psG->g DVE path work unchanged.
"""

import os
import numpy as np
from contextlib import ExitStack

import concourse.bass as bass
import concourse.tile as tile
from concourse import bacc, mybir
from concourse.bass_utils import run_bass_kernel_spmd

P = 128          # partitions
S = 4096         # sequence length
D = 1024         # input dim
H = 1024         # hidden dim
SC = 512         # seq chunk (one PSUM bank of fp32)
NSC = S // SC    # 8 seq chunks
ND = D // P      # 8 contraction subtiles for GEMM1
NH = H // P      # 8 hidden tiles
NCORES = 8

# h-GEMM fp8 subtile-pairs (each pair = 2 of the ND=8 k-subtiles).
H8P = int(os.environ.get("BIMINGRU_H8P", "1"))
NB = ND - 2 * H8P            # bf16 k-subtiles for the h-GEMM
WZ_SCALE = 32.0

F32 = mybir.dt.float32
BF16 = mybir.dt.bfloat16
F8 = mybir.dt.float8e4
O_DT = BF16                  # out-GEMM operand dtype
H_DT = BF16                  # scan output dtype (GEMM3 rhs)
DR = mybir.MatmulPerfMode.DoubleRow


def _np_dt(dt):
    import ml_dtypes
    if dt == BF16:
        return np.dtype(ml_dtypes.bfloat16)
    if dt == F8:
        return np.dtype(ml_dtypes.float8_e4m3)
    return np.dtype(np.float32)


def _build_module():
    nc = bacc.Bacc("TRN2", target_bir_lowering=False, debug=False)

    # Host-blocked inputs; one contiguous DMA per SBUF working set (the
    # sync engine's ~0.65us per-DMA issue cost would dominate the ramp):
    #   xT8 row j*128+p, col d*512+c  = x^T[d*128+p, j*512+c]   (fp8)
    #   xT16 row j*128+p, col b*512+c = x^T[(2*H8P+b)*128+p, j*512+c]
    #   Wz8  [128, NH*ND*128] fp8: [p, i*1024+d*128+c] = 32*Wz[d*128+p, i*128+c]
    #   Wh8  [128, NH*2*H8P*128] fp8: [p, i*2*H8P*128+q*128+c] = Wh[q*128+p, i*128+c]
    #   Wh16 [128, NH*NB*128] bf16: [p, i*NB*128+b*128+c] = Wh[(2*H8P+b)*128+p, i*128+c]
    #   Wo   [128, NH*H] bf16: [p, o*1024+i*128+c] = W_half[i*128+p, o*128+c]
    #   biasT [128, 4*NH] = [bz | -bz | bh | bh+0.5] per-partition columns
    xT8 = nc.dram_tensor("xT8", [NSC * P, ND * SC], F8, kind="ExternalInput").ap()
    Wz8 = nc.dram_tensor("Wz8", [P, NH * ND * P], F8, kind="ExternalInput").ap()
    if H8P > 0:
        Wh8 = nc.dram_tensor("Wh8", [P, NH * 2 * H8P * P], F8,
                             kind="ExternalInput").ap()
    if NB > 0:
        xT16 = nc.dram_tensor("xT16", [NSC * P, NB * SC], BF16,
                              kind="ExternalInput").ap()
        Wh16 = nc.dram_tensor("Wh16", [P, NH * NB * P], BF16,
                              kind="ExternalInput").ap()
    Wo = nc.dram_tensor("Wo", [P, NH * H], O_DT, kind="ExternalInput").ap()
    biasT = nc.dram_tensor("biasT", [P, 4 * NH], F32, kind="ExternalInput").ap()
    outT = nc.dram_tensor("outT", [H, S], F32, kind="ExternalOutput").ap()

    AF = mybir.ActivationFunctionType
    OP = mybir.AluOpType

    with tile.TileContext(nc) as tc, ExitStack() as ctx:
        wpool = ctx.enter_context(tc.tile_pool(name="w", bufs=1))
        xpool = ctx.enter_context(tc.tile_pool(name="x", bufs=2))
        pspool = ctx.enter_context(tc.tile_pool(name="ps", bufs=2, space="PSUM"))
        ewpool = ctx.enter_context(tc.tile_pool(name="ew", bufs=2))
        hpool = ctx.enter_context(tc.tile_pool(name="h", bufs=2))
        opool = ctx.enter_context(tc.tile_pool(name="o", bufs=3))

        # --- PE warm-up: dummy matmuls keep the PE HAM busy from t~0 so
        # the clock un-throttles (1.2 -> 2.4 GHz needs ~3.4us of sustained
        # activity) before the first real operands land (~6us).
        wdum = ewpool.tile([P, P], BF16, tag="wdum", name="wdum")
        nc.vector.memset(wdum[:], 0)
        rdum = ewpool.tile([P, SC], BF16, tag="rdum", name="rdum")
        nc.vector.memset(rdum[:], 0)
        psdum = pspool.tile([P, SC], F32, tag="psK", bufs=3, name="psdum")
        for _ in range(8):
            nc.tensor.matmul(psdum[:], wdum[:], rdum[:], start=True, stop=True)

        x8_chunks = {}
        x16_chunks = {}

        def load_x_chunk(j, split=1):
            xt8 = xpool.tile([P, ND * SC], F8, tag="x8", name=f"x8_{j}")
            qs = ND * SC // split
            for q in range(split):
                nc.sync.dma_start(xt8[:, q * qs:(q + 1) * qs],
                                  xT8[j * P:(j + 1) * P, q * qs:(q + 1) * qs])
            x8_chunks[j] = xt8
            if NB > 0:
                xt16 = xpool.tile([P, NB * SC], BF16, tag="x16", name=f"x16_{j}")
                nc.sync.dma_start(xt16[:], xT16[j * P:(j + 1) * P, :])
                x16_chunks[j] = xt16

        # Startup order: x8 chunk 0 + Wz8 are the critical path to the
        # first matmul; then bias, x16 chunk 0 + Wh for the G groups,
        # then Wo (first needed by O(0,0), ~4 i-steps into chunk 0).
        xt8 = xpool.tile([P, ND * SC], F8, tag="x8", name="x8_0")
        qs = ND * SC // 2
        nc.sync.dma_start(xt8[:, 0:qs], xT8[0:P, 0:qs])
        wz8_sb = wpool.tile([P, NH * ND * P], F8, tag="wz8", name="wz8_sb")
        nc.sync.dma_start(wz8_sb[:, 0:NH * ND * P // 2],
                          Wz8[:, 0:NH * ND * P // 2])
        nc.sync.dma_start(xt8[:, qs:2 * qs], xT8[0:P, qs:2 * qs])
        nc.sync.dma_start(wz8_sb[:, NH * ND * P // 2:],
                          Wz8[:, NH * ND * P // 2:])
        x8_chunks[0] = xt8

        bias_sb = wpool.tile([P, 4 * NH], F32, tag="bias", name="bias_sb")
        nc.sync.dma_start(bias_sb[:], biasT[:, :])
        bz_sb = bias_sb[:, 0:NH]
        nbz_sb = bias_sb[:, NH:2 * NH]
        bh_sb = bias_sb[:, 2 * NH:3 * NH]
        bh5_sb = bias_sb[:, 3 * NH:4 * NH]

        if NB > 0:
            xt16 = xpool.tile([P, NB * SC], BF16, tag="x16", name="x16_0")
            nc.sync.dma_start(xt16[:], xT16[0:P, :])
            x16_chunks[0] = xt16
        if H8P > 0:
            wh8_sb = wpool.tile([P, NH * 2 * H8P * P], F8, tag="wh8",
                                name="wh8_sb")
            nc.sync.dma_start(wh8_sb[:], Wh8[:, :])
        if NB > 0:
            wh16_sb = wpool.tile([P, NH * NB * P], BF16, tag="wh16",
                                 name="wh16_sb")
            half = NH * NB * P // 2
            nc.sync.dma_start(wh16_sb[:, 0:half], Wh16[:, 0:half])
            nc.sync.dma_start(wh16_sb[:, half:], Wh16[:, half:])

        wo_sb = wpool.tile([P, NH * H], O_DT, tag="wo", name="wo_sb")

        def load_wo():
            half = NH * H // 2
            nc.sync.dma_start(wo_sb[:, 0:half], Wo[:, 0:half])
            nc.sync.dma_start(wo_sb[:, half:], Wo[:, half:])

        # per-i weight views
        wz8_v = [wz8_sb[:, i * ND * P:(i + 1) * ND * P]
                 .rearrange("p (d c) -> p d c", c=P) for i in range(NH)]
        if H8P > 0:
            wh8_v = [wh8_sb[:, i * 2 * H8P * P:(i + 1) * 2 * H8P * P]
                     .rearrange("p (d c) -> p d c", c=P) for i in range(NH)]
        if NB > 0:
            wh16_v = [wh16_sb[:, i * NB * P:(i + 1) * NB * P]
                      for i in range(NH)]
        wo_v = [wo_sb[:, o * H:(o + 1) * H] for o in range(NH)]

        h_tiles = [[None] * NH for _ in range(NSC)]
        stash = {}

        def emit_k(j, i):
            x8v = x8_chunks[j].rearrange("p (d c) -> p d c", c=SC)
            psK = pspool.tile([P, SC], F32, tag="psK", bufs=3,
                              name=f"psK_{j}_{i}")
            for q in range(ND // 2):
                nc.tensor.matmul(
                    psK[:], wz8_v[i][:, 2 * q:2 * q + 2, :],
                    x8v[:, 2 * q:2 * q + 2, :],
                    start=(q == 0), stop=(q == ND // 2 - 1), perf_mode=DR)
            A = ewpool.tile([P, SC], F32, tag="A", bufs=3, name=f"A_{j}_{i}")
            nc.scalar.activation(A[:], psK[:], AF.Sigmoid,
                                 bias=nbz_sb[:, i:i + 1], scale=-1.0 / WZ_SCALE)
            z = ewpool.tile([P, SC], F32, tag="z", bufs=3, name=f"z_{j}_{i}")
            nc.scalar.activation(z[:], psK[:], AF.Sigmoid,
                                 bias=bz_sb[:, i:i + 1], scale=1.0 / WZ_SCALE)
            stash[(j, i)] = (A, z)

        def emit_g(j, i):
            psG = pspool.tile([P, SC], F32, tag="psG", bufs=3,
                              name=f"psG_{j}_{i}")
            if H8P > 0:
                x8v = x8_chunks[j].rearrange("p (d c) -> p d c", c=SC)
                for q in range(H8P):
                    nc.tensor.matmul(
                        psG[:], wh8_v[i][:, 2 * q:2 * q + 2, :],
                        x8v[:, 2 * q:2 * q + 2, :],
                        start=(q == 0), stop=(NB == 0 and q == H8P - 1),
                        perf_mode=DR)
            if NB > 0:
                x16c = x16_chunks[j]
                for b in range(NB):
                    nc.tensor.matmul(
                        psG[:], wh16_v[i][:, b * P:(b + 1) * P],
                        x16c[:, b * SC:(b + 1) * SC],
                        start=(H8P == 0 and b == 0), stop=(b == NB - 1))
            A, z = stash.pop((j, i))
            sg = ewpool.tile([P, SC], F32, tag="sg", name=f"sg_{j}_{i}")
            nc.scalar.activation(sg[:], psG[:], AF.Sigmoid,
                                 bias=bh_sb[:, i:i + 1], scale=1.0)
            g = ewpool.tile([P, SC], F32, tag="g", name=f"g_{j}_{i}")
            nc.vector.scalar_tensor_tensor(g[:], psG[:], bh5_sb[:, i:i + 1],
                                           sg[:], op0=OP.add, op1=OP.max)
            Bv = ewpool.tile([P, SC], F32, tag="B", name=f"B_{j}_{i}")
            nc.vector.tensor_tensor(Bv[:], z[:], g[:], op=OP.mult)

            ht = hpool.tile([P, SC], H_DT, tag=f"h{i}", name=f"h_{j}_{i}")
            init = 0.0 if j == 0 else h_tiles[j - 1][i][:, SC - 1:SC]
            nc.vector.tensor_tensor_scan(ht[:], A[:], Bv[:], initial=init,
                                         op0=OP.mult, op1=OP.add)
            h_tiles[j][i] = ht

        TAIL_TAGS = ["psO", "psK", "psG"]
        TAIL_BUFS = {"psO": 2, "psK": 3, "psG": 3}

        def emit_o(j, o, tail=False):
            # In the drain tail there is no K/G work between O groups, so a
            # 2-deep psO rotation + a single ACT copy chain stalls the PE
            # ~1.7us per group. The psK/psG banks are free there: rotate
            # across all 8 banks and alternate the PSUM->SBUF copy between
            # ACT and DVE so the copies keep up.
            tag = TAIL_TAGS[o % 3] if tail else "psO"
            psO = pspool.tile([P, SC], F32, tag=tag, bufs=TAIL_BUFS[tag],
                              name=f"psO_{j}_{o}")
            for i in range(NH):
                nc.tensor.matmul(
                    psO[:], wo_v[o][:, i * P:(i + 1) * P],
                    h_tiles[j][i][:],
                    start=(i == 0), stop=(i == NH - 1))
            oc = opool.tile([P, SC], F32, tag="oc", bufs=6, name=f"oc_{j}_{o}")
            if tail and o % 2 == 1:
                nc.vector.tensor_copy(oc[:], psO[:])
            else:
                nc.scalar.copy(oc[:], psO[:])
            nc.sync.dma_start(outT[o * P:(o + 1) * P, j * SC:(j + 1) * SC], oc[:])

        # Software pipeline. Per chunk j the PE group order is
        #   K0 K1 [G0 O0] [K2 G1 O1] ... [K7 G6 O6] [G7 O7]
        # where O* are the GEMM3 groups of chunk j-1 (GEMM3 contracts over
        # all NH h-tiles, so it can only start after the whole previous
        # chunk's scans). x(j+1) is prefetched at the head of chunk j.
        for j in range(NSC):
            if j == 0:
                load_wo()
            if j + 1 < NSC:
                load_x_chunk(j + 1)
            emit_k(j, 0)
            emit_k(j, 1)
            for i in range(NH):
                if i + 2 < NH:
                    emit_k(j, i + 2)
                emit_g(j, i)
                if j >= 1:
                    emit_o(j - 1, i)
        for o in range(NH - 1):
            emit_o(NSC - 1, o, tail=True)
        # final O group split into two N=256 halves so the first half's
        # copy+store overlaps the second half's matmuls (shorter serial
        # tail before the drain barrier)
        j, o = NSC - 1, NH - 1
        HC = SC // 2
        for half in range(2):
            tag = TAIL_TAGS[(NH - 1 + half) % 3]
            psO = pspool.tile([P, HC], F32, tag=tag, bufs=TAIL_BUFS[tag],
                              name=f"psOt_{half}")
            for i in range(NH):
                nc.tensor.matmul(
                    psO[:], wo_v[o][:, i * P:(i + 1) * P],
                    h_tiles[j][i][:, half * HC:(half + 1) * HC],
                    start=(i == 0), stop=(i == NH - 1))
            oc = opool.tile([P, HC], F32, tag="oc", bufs=6, name=f"oct_{half}")
            if half == 0:
                nc.scalar.copy(oc[:], psO[:])
            else:
                nc.vector.tensor_copy(oc[:], psO[:])
            nc.sync.dma_start(
                outT[o * P:(o + 1) * P,
                     j * SC + half * HC:j * SC + (half + 1) * HC], oc[:])

    nc.compile()
    return nc


_CACHE = {}


def _get_module():
    if "nc" not in _CACHE:
        _CACHE["nc"] = _build_module()
    return _CACHE["nc"]


def _make_in_maps(x, Wz_f, bz_f, Wh_f, bh_f, Wz_b, bz_b, Wh_b, bh_b, W_out, b_out):
    np_f8 = _np_dt(F8)
    np_bf = _np_dt(BF16)
    np_o = _np_dt(O_DT)
    f32 = np.float32

    def blk_w(w, dt, scale=1.0):
        # [D, H] -> [128, NH*ND*128]: out[p, i*D + d*128 + c] = w[d*128+p, i*128+c]
        w = np.asarray(w, dtype=f32)
        if scale != 1.0:
            w = w * scale
        w = w.reshape(ND, P, NH, P)
        # target axes order: (p, i, d, c)
        return np.ascontiguousarray(
            w.transpose(1, 2, 0, 3).reshape(P, NH * ND * P), dtype=dt)

    def blk_w_rows(w, dt, d_lo, d_hi):
        # rows d_lo*128..d_hi*128 of [D, H] -> [128, NH*(d_hi-d_lo)*128]
        nd = d_hi - d_lo
        w = np.asarray(w[d_lo * P:d_hi * P], dtype=f32).reshape(nd, P, NH, P)
        return np.ascontiguousarray(
            w.transpose(1, 2, 0, 3).reshape(P, NH * nd * P), dtype=dt)

    def blk_x(xb, rev, d_lo, d_hi, dt):
        # [S, D] cols d_lo*128..d_hi*128 -> [NSC*128, nd*512]:
        # out[j*128+p, b*512+c] = x[j*512+c, (d_lo+b)*128+p]
        nd = d_hi - d_lo
        if rev:
            xb = xb[::-1]
        xb = xb[:, d_lo * P:d_hi * P].reshape(NSC, SC, nd, P)
        return np.ascontiguousarray(
            xb.transpose(0, 3, 2, 1).reshape(NSC * P, nd * SC), dtype=dt)

    def blk_wo(w, dt):
        # [H, H] -> [128, NH*H]: out[p, o*H + i*128 + c] = w[i*128+p, o*128+c]
        w = np.asarray(w, dtype=f32).reshape(NH, P, NH, P)
        return np.ascontiguousarray(
            w.transpose(1, 2, 0, 3).reshape(P, NH * H), dtype=dt)

    x = np.asarray(x, dtype=f32)
    W_out = np.asarray(W_out)

    def weights(Wz, Wh):
        m = {"Wz8": blk_w(Wz, np_f8, scale=WZ_SCALE)}
        if H8P > 0:
            m["Wh8"] = blk_w_rows(Wh, np_f8, 0, 2 * H8P)
        if NB > 0:
            m["Wh16"] = blk_w_rows(Wh, np_bf, 2 * H8P, ND)
        return m

    w_f = weights(Wz_f, Wh_f)
    w_b = weights(Wz_b, Wh_b)
    wo_f = blk_wo(W_out[:H], np_o)      # fwd half rows of W_out
    wo_b = blk_wo(W_out[H:], np_o)      # bwd half rows

    def bias_pack(b_z, b_h):
        def col(v):  # [H] -> [128, NH] with col i = h-tile i
            return np.asarray(v, dtype=f32).reshape(NH, P).T
        b_z = np.asarray(b_z, dtype=f32)
        b_h = np.asarray(b_h, dtype=f32)
        return np.ascontiguousarray(np.concatenate(
            [col(b_z), col(-b_z), col(b_h), col(b_h + 0.5)], axis=1))

    bias_f = bias_pack(bz_f, bh_f)
    bias_b = bias_pack(bz_b, bh_b)

    in_maps = []
    for b in range(4):
        for rev, wd, wo, bias in ((False, w_f, wo_f, bias_f),
                                  (True, w_b, wo_b, bias_b)):
            m = {"xT8": blk_x(x[b], rev, 0, ND, np_f8),
                 "Wo": wo, "biasT": bias, **wd}
            if NB > 0:
                m["xT16"] = blk_x(x[b], rev, 2 * H8P, ND, np_bf)
            in_maps.append(m)
    return in_maps


def _assemble(results, b_out):
    out = np.empty((4, S, H), np.float32)
    for b in range(4):
        out[b] = results[2 * b]["outT"].T
        out[b] += results[2 * b + 1]["outT"].T
    out += np.asarray(b_out, dtype=np.float32)
    return out


def kernel(x, Wz_f, bz_f, Wh_f, bh_f, Wz_b, bz_b, Wh_b, bh_b, W_out, b_out):
    nc = _get_module()
    in_maps = _make_in_maps(x, Wz_f, bz_f, Wh_f, bh_f,
                            Wz_b, bz_b, Wh_b, bh_b, W_out, b_out)
    res = run_bass_kernel_spmd(nc, in_maps, core_ids=list(range(NCORES)))
    return _assemble(res.results, b_out)


# revision 15
# speedup vs baseline: 1.2345x; 1.0031x over previous
"""Bi-directional MinGRU kernel for Trainium2 (8 NeuronCores, SPMD).

Problem: x [4, 4096, 1024]; per direction d in {fwd, bwd}:
    k  = x @ Wz_d + bz_d
    A  = sigmoid(-k)           (= 1 - z, the carry coefficient)
    z  = sigmoid(k)
    gp = x @ Wh_d + bh_d
    g  = max(gp + 0.5, sigmoid(gp))      (== where(gp>=0, gp+0.5, sigmoid(gp)))
    h_t = A_t * h_{t-1} + z_t * g_t      (linear first-order scan over S)
    out = concat(h_fwd, h_bwd) @ W_out + b_out

Sharding: 8 cores = (4 batches) x (2 directions). Each core computes the
full hidden state for one (batch, direction) and its half of the final
2H->H projection; the two partial products per batch are summed on host.

Per-core layout: everything is kept transposed ([channel, seq]) so the
sequential scan runs along the free dimension with channels on partitions,
using the native VectorE tensor_tensor_scan instruction.

Precision plan (error budget, gate rel < 2e-2; all rels are exact
host-side replications of the device arithmetic, see fp8_sweep.py):
  - z-GEMM (k = x@Wz) fully fp8-e4m3 in DoubleRow mode (2 k-subtiles per
    matmul at ~1.77x bf16 rate). The z errors are strongly damped: the
    sigmoid slope (<=1/4) and the scan innovation form dh = dz*(g - h)
    cancel most of it. Contribution ~8.5e-3.
  - h-GEMM (gp = x@Wh): H8P k-subtile PAIRS in fp8 DoubleRow, the rest
    bf16, accumulated into the same PSUM tile. g-errors pass slope-1
    through max(gp+.5, sg), so only a fraction of K may be fp8.
  - out-GEMM (h@Wo) stays bf16: its error hits the output undamped
    (fp8 would contribute ~4.1e-2 alone).
  - Wz is host-scaled by 32 (=> fewer e4m3 subnormal flushes); folded
    back via the ACT sigmoid scale parameter. x and Wh stay unscaled so
    the shared x8 tile and the un-scaled psG->g DVE path work unchanged.
"""

import os
import numpy as np
from contextlib import ExitStack

import concourse.bass as bass
import concourse.tile as tile
from concourse import bacc, mybir
from concourse.bass_utils import run_bass_kernel_spmd

P = 128          # partitions
S = 4096         # sequence length
D = 1024         # input dim
H = 1024         # hidden dim
SC = 512         # seq chunk (one PSUM bank of fp32)
NSC = S // SC    # 8 seq chunks
ND = D // P      # 8 contraction subtiles for GEMM1
NH = H // P      # 8 hidden tiles
NCORES = 8

# h-GEMM fp8 subtile-pairs (each pair = 2 of the ND=8 k-subtiles).
H8P = int(os.environ.get("BIMINGRU_H8P", "1"))
NB = ND - 2 * H8P            # bf16 k-subtiles for the h-GEMM
WZ_SCALE = 32.0

F32 = mybir.dt.float32
BF16 = mybir.dt.bfloat16
F8 = mybir.dt.float8e4
O_DT = BF16                  # out-GEMM operand dtype
H_DT = BF16                  # scan output dtype (GEMM3 rhs)
DR = mybir.MatmulPerfMode.DoubleRow


def _np_dt(dt):
    import ml_dtypes
    if dt == BF16:
        return np.dtype(ml_dtypes.bfloat16)
    if dt == F8:
        return np.dtype(ml_dtypes.float8_e4m3)
    return np.dtype(np.float32)


def _build_module():
    nc = bacc.Bacc("TRN2", target_bir_lowering=False, debug=False)

    # Host-blocked inputs; one contiguous DMA per SBUF working set (the
    # sync engine's ~0.65us per-DMA issue cost would dominate the ramp):
    #   xT8 row j*128+p, col d*512+c  = x^T[d*128+p, j*512+c]   (fp8)
    #   xT16 row j*128+p, col b*512+c = x^T[(2*H8P+b)*128+p, j*512+c]
    #   Wz8  [128, NH*ND*128] fp8: [p, i*1024+d*128+c] = 32*Wz[d*128+p, i*128+c]
    #   Wh8  [128, NH*2*H8P*128] fp8: [p, i*2*H8P*128+q*128+c] = Wh[q*128+p, i*128+c]
    #   Wh16 [128, NH*NB*128] bf16: [p, i*NB*128+b*128+c] = Wh[(2*H8P+b)*128+p, i*128+c]
    #   Wo   [128, NH*H] bf16: [p, o*1024+i*128+c] = W_half[i*128+p, o*128+c]
    #   biasT [128, 4*NH] = [bz | -bz | bh | bh+0.5] per-partition columns
    xT8 = nc.dram_tensor("xT8", [NSC * P, ND * SC], F8, kind="ExternalInput").ap()
    Wz8 = nc.dram_tensor("Wz8", [P, NH * ND * P], F8, kind="ExternalInput").ap()
    if H8P > 0:
        Wh8 = nc.dram_tensor("Wh8", [P, NH * 2 * H8P * P], F8,
                             kind="ExternalInput").ap()
    if NB > 0:
        xT16 = nc.dram_tensor("xT16", [NSC * P, NB * SC], BF16,
                              kind="ExternalInput").ap()
        Wh16 = nc.dram_tensor("Wh16", [P, NH * NB * P], BF16,
                              kind="ExternalInput").ap()
    Wo = nc.dram_tensor("Wo", [P, NH * H], O_DT, kind="ExternalInput").ap()
    biasT = nc.dram_tensor("biasT", [P, 4 * NH], F32, kind="ExternalInput").ap()
    outT = nc.dram_tensor("outT", [H, S], F32, kind="ExternalOutput").ap()

    AF = mybir.ActivationFunctionType
    OP = mybir.AluOpType

    with tile.TileContext(nc) as tc, ExitStack() as ctx:
        wpool = ctx.enter_context(tc.tile_pool(name="w", bufs=1))
        xpool = ctx.enter_context(tc.tile_pool(name="x", bufs=2))
        pspool = ctx.enter_context(tc.tile_pool(name="ps", bufs=2, space="PSUM"))
        ewpool = ctx.enter_context(tc.tile_pool(name="ew", bufs=2))
        hpool = ctx.enter_context(tc.tile_pool(name="h", bufs=2))
        opool = ctx.enter_context(tc.tile_pool(name="o", bufs=3))

        # --- PE warm-up: dummy matmuls keep the PE HAM busy from ~1.9us
        # (DVE init + memsets) so the clock un-throttles (1.2 -> 2.4 GHz
        # needs ~3.4us of sustained activity) around when the first real
        # operands land (~3.5us). 6 dummies pace ~0.55us each
        # (WAW-serialized), ending ~5.2us.
        wdum = ewpool.tile([P, P], BF16, tag="wdum", name="wdum")
        nc.vector.memset(wdum[:], 0)
        rdum = ewpool.tile([P, SC], BF16, tag="rdum", name="rdum")
        nc.vector.memset(rdum[:], 0)
        psdum = pspool.tile([P, SC], F32, tag="psK", bufs=3, name="psdum")
        for _ in range(6):
            nc.tensor.matmul(psdum[:], wdum[:], rdum[:], start=True, stop=True)

        x8_chunks = {}
        x16_chunks = {}

        def load_x_chunk(j, split=1):
            xt8 = xpool.tile([P, ND * SC], F8, tag="x8", name=f"x8_{j}")
            qs = ND * SC // split
            for q in range(split):
                nc.sync.dma_start(xt8[:, q * qs:(q + 1) * qs],
                                  xT8[j * P:(j + 1) * P, q * qs:(q + 1) * qs])
            x8_chunks[j] = xt8
            if NB > 0:
                xt16 = xpool.tile([P, NB * SC], BF16, tag="x16", name=f"x16_{j}")
                nc.sync.dma_start(xt16[:], xT16[j * P:(j + 1) * P, :])
                x16_chunks[j] = xt16

        # Startup order: x8 chunk 0 + Wz8 are the critical path to the
        # first matmuls (the chunk-0 prelude is z-GEMMs only); then bias,
        # Wh + x16 chunk 0 for the G groups (needed ~8us in), then Wo
        # (first needed by O(0,*) during chunk 1).
        xt8 = xpool.tile([P, ND * SC], F8, tag="x8", name="x8_0")
        qs = ND * SC // 2
        nc.sync.dma_start(xt8[:, 0:qs], xT8[0:P, 0:qs])
        wz8_sb = wpool.tile([P, NH * ND * P], F8, tag="wz8", name="wz8_sb")
        nc.sync.dma_start(wz8_sb[:, 0:NH * ND * P // 2],
                          Wz8[:, 0:NH * ND * P // 2])
        x8_chunks[0] = xt8

        bias_sb = wpool.tile([P, 4 * NH], F32, tag="bias", name="bias_sb")
        nc.sync.dma_start(bias_sb[:], biasT[:, :])
        bz_sb = bias_sb[:, 0:NH]
        nbz_sb = bias_sb[:, NH:2 * NH]
        bh_sb = bias_sb[:, 2 * NH:3 * NH]
        bh5_sb = bias_sb[:, 3 * NH:4 * NH]

        nc.sync.dma_start(xt8[:, qs:2 * qs], xT8[0:P, qs:2 * qs])
        nc.sync.dma_start(wz8_sb[:, NH * ND * P // 2:],
                          Wz8[:, NH * ND * P // 2:])

        if H8P > 0:
            wh8_sb = wpool.tile([P, NH * 2 * H8P * P], F8, tag="wh8",
                                name="wh8_sb")
            nc.sync.dma_start(wh8_sb[:], Wh8[:, :])
        if NB > 0:
            xt16 = xpool.tile([P, NB * SC], BF16, tag="x16", name="x16_0")
            nc.sync.dma_start(xt16[:], xT16[0:P, :])
            x16_chunks[0] = xt16
            wh16_sb = wpool.tile([P, NH * NB * P], BF16, tag="wh16",
                                 name="wh16_sb")
            half = NH * NB * P // 2
            nc.sync.dma_start(wh16_sb[:, 0:half], Wh16[:, 0:half])
            nc.sync.dma_start(wh16_sb[:, half:], Wh16[:, half:])

        wo_sb = wpool.tile([P, NH * H], O_DT, tag="wo", name="wo_sb")

        def load_wo():
            half = NH * H // 2
            nc.sync.dma_start(wo_sb[:, 0:half], Wo[:, 0:half])
            nc.sync.dma_start(wo_sb[:, half:], Wo[:, half:])

        # per-i weight views
        wz8_v = [wz8_sb[:, i * ND * P:(i + 1) * ND * P]
                 .rearrange("p (d c) -> p d c", c=P) for i in range(NH)]
        if H8P > 0:
            wh8_v = [wh8_sb[:, i * 2 * H8P * P:(i + 1) * 2 * H8P * P]
                     .rearrange("p (d c) -> p d c", c=P) for i in range(NH)]
        if NB > 0:
            wh16_v = [wh16_sb[:, i * NB * P:(i + 1) * NB * P]
                      for i in range(NH)]
        wo_v = [wo_sb[:, o * H:(o + 1) * H] for o in range(NH)]

        h_tiles = [[None] * NH for _ in range(NSC)]
        stash = {}

        K_TAGS = ["psK", "psG", "psO"]
        K_BUFS = {"psK": 3, "psG": 3, "psO": 2}

        def emit_k(j, i, prelude=False):
            x8v = x8_chunks[j].rearrange("p (d c) -> p d c", c=SC)
            # During the chunk-0 z-only prelude the psG/psO banks are idle;
            # rotate the 8 K groups across all 8 PSUM banks so the prelude
            # never stalls on the ACT A/z drain.
            tag = K_TAGS[i % 3] if prelude else "psK"
            psK = pspool.tile([P, SC], F32, tag=tag, bufs=K_BUFS[tag],
                              name=f"psK_{j}_{i}")
            for q in range(ND // 2):
                nc.tensor.matmul(
                    psK[:], wz8_v[i][:, 2 * q:2 * q + 2, :],
                    x8v[:, 2 * q:2 * q + 2, :],
                    start=(q == 0), stop=(q == ND // 2 - 1), perf_mode=DR)
            # bufs=9: the chunk-0 prelude produces 8 A/z tiles before the
            # first scan consumes any (fewer bufs would deadlock the
            # prelude through the psK -> ACT -> scan chain).
            A = ewpool.tile([P, SC], F32, tag="A", bufs=9, name=f"A_{j}_{i}")
            nc.scalar.activation(A[:], psK[:], AF.Sigmoid,
                                 bias=nbz_sb[:, i:i + 1], scale=-1.0 / WZ_SCALE)
            z = ewpool.tile([P, SC], F32, tag="z", bufs=9, name=f"z_{j}_{i}")
            nc.scalar.activation(z[:], psK[:], AF.Sigmoid,
                                 bias=bz_sb[:, i:i + 1], scale=1.0 / WZ_SCALE)
            stash[(j, i)] = (A, z)

        def emit_g(j, i):
            psG = pspool.tile([P, SC], F32, tag="psG", bufs=3,
                              name=f"psG_{j}_{i}")
            if H8P > 0:
                x8v = x8_chunks[j].rearrange("p (d c) -> p d c", c=SC)
                for q in range(H8P):
                    nc.tensor.matmul(
                        psG[:], wh8_v[i][:, 2 * q:2 * q + 2, :],
                        x8v[:, 2 * q:2 * q + 2, :],
                        start=(q == 0), stop=(NB == 0 and q == H8P - 1),
                        perf_mode=DR)
            if NB > 0:
                x16c = x16_chunks[j]
                for b in range(NB):
                    nc.tensor.matmul(
                        psG[:], wh16_v[i][:, b * P:(b + 1) * P],
                        x16c[:, b * SC:(b + 1) * SC],
                        start=(H8P == 0 and b == 0), stop=(b == NB - 1))
            A, z = stash.pop((j, i))
            sg = ewpool.tile([P, SC], F32, tag="sg", name=f"sg_{j}_{i}")
            nc.scalar.activation(sg[:], psG[:], AF.Sigmoid,
                                 bias=bh_sb[:, i:i + 1], scale=1.0)
            g = ewpool.tile([P, SC], F32, tag="g", name=f"g_{j}_{i}")
            nc.vector.scalar_tensor_tensor(g[:], psG[:], bh5_sb[:, i:i + 1],
                                           sg[:], op0=OP.add, op1=OP.max)
            Bv = ewpool.tile([P, SC], F32, tag="B", name=f"B_{j}_{i}")
            nc.vector.tensor_tensor(Bv[:], z[:], g[:], op=OP.mult)

            ht = hpool.tile([P, SC], H_DT, tag=f"h{i}", name=f"h_{j}_{i}")
            init = 0.0 if j == 0 else h_tiles[j - 1][i][:, SC - 1:SC]
            nc.vector.tensor_tensor_scan(ht[:], A[:], Bv[:], initial=init,
                                         op0=OP.mult, op1=OP.add)
            h_tiles[j][i] = ht

        TAIL_TAGS = ["psO", "psK", "psG"]
        TAIL_BUFS = {"psO": 2, "psK": 3, "psG": 3}

        def emit_o(j, o, tail=False):
            # In the drain tail there is no K/G work between O groups, so a
            # 2-deep psO rotation + a single ACT copy chain stalls the PE
            # ~1.7us per group. The psK/psG banks are free there: rotate
            # across all 8 banks and alternate the PSUM->SBUF copy between
            # ACT and DVE so the copies keep up.
            tag = TAIL_TAGS[o % 3] if tail else "psO"
            psO = pspool.tile([P, SC], F32, tag=tag, bufs=TAIL_BUFS[tag],
                              name=f"psO_{j}_{o}")
            for i in range(NH):
                nc.tensor.matmul(
                    psO[:], wo_v[o][:, i * P:(i + 1) * P],
                    h_tiles[j][i][:],
                    start=(i == 0), stop=(i == NH - 1))
            oc = opool.tile([P, SC], F32, tag="oc", bufs=6, name=f"oc_{j}_{o}")
            if tail and o % 2 == 1:
                nc.vector.tensor_copy(oc[:], psO[:])
            else:
                nc.scalar.copy(oc[:], psO[:])
            nc.sync.dma_start(outT[o * P:(o + 1) * P, j * SC:(j + 1) * SC], oc[:])

        # Software pipeline. Per chunk j the PE group order is
        #   K0 K1 [G0 O0] [K2 G1 O1] ... [K7 G6 O6] [G7 O7]
        # where O* are the GEMM3 groups of chunk j-1 (GEMM3 contracts over
        # all NH h-tiles, so it can only start after the whole previous
        # chunk's scans). x(j+1) is prefetched at the head of chunk j.
        # Software pipeline, K running 4 groups ahead of G. Chunk 0 opens
        # with a z-only prelude of 4 K groups (spread across all 8 PSUM
        # banks) so the PE has work while the Wh/x16/Wo DMAs stream in;
        # the same 4-ahead offset then holds for the whole kernel:
        #   K0 K1 K2 K3 [K4 G0] [K5 G1] ... [K'0 G4 O..] [K'1 G5] ...
        # O* are the GEMM3 groups of chunk j-1 (GEMM3 contracts over all
        # NH h-tiles, so it can only start after the whole previous
        # chunk's scans). x(j+1) is prefetched at the head of chunk j.
        load_wo()
        for i in range(4):
            emit_k(0, i, prelude=True)
        for j in range(NSC):
            if j + 1 < NSC:
                load_x_chunk(j + 1)
            for i in range(NH):
                kk = i + 4
                if kk < NH:
                    emit_k(j, kk)
                elif j + 1 < NSC:
                    emit_k(j + 1, kk - NH)
                emit_g(j, i)
                if j >= 1:
                    emit_o(j - 1, i)
        for o in range(NH - 1):
            emit_o(NSC - 1, o, tail=True)
        # final O group split into two N=256 halves so the first half's
        # copy+store overlaps the second half's matmuls (shorter serial
        # tail before the drain barrier)
        j, o = NSC - 1, NH - 1
        HC = SC // 2
        for half in range(2):
            tag = TAIL_TAGS[(NH - 1 + half) % 3]
            psO = pspool.tile([P, HC], F32, tag=tag, bufs=TAIL_BUFS[tag],
                              name=f"psOt_{half}")
            for i in range(NH):
                nc.tensor.matmul(
                    psO[:], wo_v[o][:, i * P:(i + 1) * P],
                    h_tiles[j][i][:, half * HC:(half + 1) * HC],
                    start=(i == 0), stop=(i == NH - 1))
            oc = opool.tile([P, HC], F32, tag="oc", bufs=6, name=f"oct_{half}")
            if half == 0:
                nc.scalar.copy(oc[:], psO[:])
            else:
                nc.vector.tensor_copy(oc[:], psO[:])
            nc.sync.dma_start(
                outT[o * P:(o + 1) * P,
                     j * SC + half * HC:j * SC + (half + 1) * HC], oc[:])

    nc.compile()
    return nc


_CACHE = {}


def _get_module():
    if "nc" not in _CACHE:
        _CACHE["nc"] = _build_module()
    return _CACHE["nc"]


def _make_in_maps(x, Wz_f, bz_f, Wh_f, bh_f, Wz_b, bz_b, Wh_b, bh_b, W_out, b_out):
    np_f8 = _np_dt(F8)
    np_bf = _np_dt(BF16)
    np_o = _np_dt(O_DT)
    f32 = np.float32

    def blk_w(w, dt, scale=1.0):
        # [D, H] -> [128, NH*ND*128]: out[p, i*D + d*128 + c] = w[d*128+p, i*128+c]
        w = np.asarray(w, dtype=f32)
        if scale != 1.0:
            w = w * scale
        w = w.reshape(ND, P, NH, P)
        # target axes order: (p, i, d, c)
        return np.ascontiguousarray(
            w.transpose(1, 2, 0, 3).reshape(P, NH * ND * P), dtype=dt)

    def blk_w_rows(w, dt, d_lo, d_hi):
        # rows d_lo*128..d_hi*128 of [D, H] -> [128, NH*(d_hi-d_lo)*128]
        nd = d_hi - d_lo
        w = np.asarray(w[d_lo * P:d_hi * P], dtype=f32).reshape(nd, P, NH, P)
        return np.ascontiguousarray(
            w.transpose(1, 2, 0, 3).reshape(P, NH * nd * P), dtype=dt)

    def blk_x(xb, rev, d_lo, d_hi, dt):
        # [S, D] cols d_lo*128..d_hi*128 -> [NSC*128, nd*512]:
        # out[j*128+p, b*512+c] = x[j*512+c, (d_lo+b)*128+p]
        nd = d_hi - d_lo
        if rev:
            xb = xb[::-1]
        xb = xb[:, d_lo * P:d_hi * P].reshape(NSC, SC, nd, P)
        return np.ascontiguousarray(
            xb.transpose(0, 3, 2, 1).reshape(NSC * P, nd * SC), dtype=dt)

    def blk_wo(w, dt):
        # [H, H] -> [128, NH*H]: out[p, o*H + i*128 + c] = w[i*128+p, o*128+c]
        w = np.asarray(w, dtype=f32).reshape(NH, P, NH, P)
        return np.ascontiguousarray(
            w.transpose(1, 2, 0, 3).reshape(P, NH * H), dtype=dt)

    x = np.asarray(x, dtype=f32)
    W_out = np.asarray(W_out)

    def weights(Wz, Wh):
        m = {"Wz8": blk_w(Wz, np_f8, scale=WZ_SCALE)}
        if H8P > 0:
            m["Wh8"] = blk_w_rows(Wh, np_f8, 0, 2 * H8P)
        if NB > 0:
            m["Wh16"] = blk_w_rows(Wh, np_bf, 2 * H8P, ND)
        return m

    w_f = weights(Wz_f, Wh_f)
    w_b = weights(Wz_b, Wh_b)
    wo_f = blk_wo(W_out[:H], np_o)      # fwd half rows of W_out
    wo_b = blk_wo(W_out[H:], np_o)      # bwd half rows

    def bias_pack(b_z, b_h):
        def col(v):  # [H] -> [128, NH] with col i = h-tile i
            return np.asarray(v, dtype=f32).reshape(NH, P).T
        b_z = np.asarray(b_z, dtype=f32)
        b_h = np.asarray(b_h, dtype=f32)
        return np.ascontiguousarray(np.concatenate(
            [col(b_z), col(-b_z), col(b_h), col(b_h + 0.5)], axis=1))

    bias_f = bias_pack(bz_f, bh_f)
    bias_b = bias_pack(bz_b, bh_b)

    in_maps = []
    for b in range(4):
        for rev, wd, wo, bias in ((False, w_f, wo_f, bias_f),
                                  (True, w_b, wo_b, bias_b)):
            m = {"xT8": blk_x(x[b], rev, 0, ND, np_f8),
                 "Wo": wo, "biasT": bias, **wd}
            if NB > 0:
                m["xT16"] = blk_x(x[b], rev, 2 * H8P, ND, np_bf)
            in_maps.append(m)
    return in_maps


def _assemble(results, b_out):
    out = np.empty((4, S, H), np.float32)
    for b in range(4):
        out[b] = results[2 * b]["outT"].T
        out[b] += results[2 * b + 1]["outT"].T
    out += np.asarray(b_out, dtype=np.float32)
    return out


def kernel(x, Wz_f, bz_f, Wh_f, bh_f, Wz_b, bz_b, Wh_b, bh_b, W_out, b_out):
    nc = _get_module()
    in_maps = _make_in_maps(x, Wz_f, bz_f, Wh_f, bh_f,
                            Wz_b, bz_b, Wh_b, bh_b, W_out, b_out)
    res = run_bass_kernel_spmd(nc, in_maps, core_ids=list(range(NCORES)))
    return _assemble(res.results, b_out)
